# revision 5
# baseline (speedup 1.0000x reference)
"""Causal self-attention (B=2, T=2048, C=1024, H=16, D=64) on 8 TRN2 NeuronCores.

Sharding: batch x head-group. Core i handles batch b=i//4 and heads
[4*(i%4) .. 4*(i%4)+3]. c_attn is column-parallel (each core computes q,k,v
only for its 4 heads), c_proj is row-parallel (each core produces a partial
[T, C] output; the all-reduce over the 4 cores of a batch happens on the host
during unsharding).

Per-core kernel layout (all matmul inputs bf16, PSUM accumulation fp32):
  xT[cb]  [128, 2048]  x^T per 128-wide channel block (DMA xbar transpose)
  qT/kT   [128, 2048]  per head-pair (h0 rows 0:64, h1 rows 64:128)
  v_aug   [128, 16, 4, 65]  v tiles [t2-tile, head] with a ones column (row
                            sums of attn emerge as row 64 of the AV psum)
  scoresT = kT.T-tile @ qT-chunk -> [128 t2, 512 t1] psum, exp on ACT,
  causal mask as 0/1 multiply on DVE (diagonal tiles only),
  yT_aug = v_aug.T @ attnT accumulated over t2, normalize by row 64,
  out_partial = yT.T-tile @ w_proj rows.
"""

import sys

sys.path.insert(0, "/opt/trn_rl_repo")

import numpy as np
import ml_dtypes

import concourse.bacc as bacc
import concourse.mybir as mybir
import concourse.tile as tile
from concourse.bass_utils import run_bass_kernel_spmd

BF16 = mybir.dt.bfloat16
F32 = mybir.dt.float32

B, T, C = 2, 2048, 1024
H, D = 16, 64
HEADS_PER_CORE = 4
N_CORES = 8
P = 128
TCH = 512            # t1 moving-chunk width
NT1 = T // TCH       # 4 t1 chunks
NT2 = T // P         # 16 t2 tiles
NCB = C // P         # 8 channel blocks


def build_program():
    nc = bacc.Bacc("TRN2", target_bir_lowering=False, debug=False)
    # x arrives pre-transposed from the host: x[c, t]
    x = nc.declare_dram_parameter("x", [C, T], BF16, isOutput=False)
    wqk = nc.declare_dram_parameter("wqk", [C, 512], BF16, isOutput=False)
    wv = nc.declare_dram_parameter("wv", [C, 256], BF16, isOutput=False)
    wp = nc.declare_dram_parameter("wp", [256, C], BF16, isOutput=False)
    out = nc.declare_dram_parameter("out", [T, C], F32, isOutput=True)

    with tile.TileContext(nc) as tc:
        build_body(nc, tc, x, wqk, wv, wp, out)
    nc.compile()
    return nc


def build_body(nc, tc, x, wqk, wv, wp, out):
    from contextlib import ExitStack

    ctx = ExitStack()
    pers = ctx.enter_context(tc.tile_pool(name="pers", bufs=1))
    work = ctx.enter_context(tc.tile_pool(name="work", bufs=2))
    attn_pool = ctx.enter_context(tc.tile_pool(name="attn", bufs=44))
    psum = ctx.enter_context(tc.tile_pool(name="psum", bufs=2, space="PSUM"))

    # ---- persistent SBUF tensors ----
    xT = [pers.tile([P, T], BF16, tag=f"xT{cb}", name=f"xT{cb}") for cb in range(NCB)]
    wqk_sb = pers.tile([P, NCB, 512], BF16, tag="wqk", name="wqk_sb")
    wv_sb = pers.tile([P, NCB, 256], BF16, tag="wv", name="wv_sb")
    wp_sb = pers.tile([P, 2, C], BF16, tag="wp", name="wp_sb")
    # q/k transposed, head pairs stacked on partitions
    qT = [pers.tile([P, T], BF16, tag=f"qT{i}", name=f"qT{i}") for i in range(2)]
    kT = [pers.tile([P, T], BF16, tag=f"kT{i}", name=f"kT{i}") for i in range(2)]
    v_aug = pers.tile([P, NT2, HEADS_PER_CORE, D + 1], BF16, tag="vaug", name="v_aug")
    yT = [pers.tile([P, T], BF16, tag=f"yT{i}", name=f"yT{i}") for i in range(2)]
    maskf = pers.tile([P, 896], BF16, tag="mask", name="maskf")

    # ---- input DMAs ----
    for cb in range(NCB):
        nc.sync.dma_start(out=xT[cb][:], in_=x[cb * P:(cb + 1) * P, :])
    nc.sync.dma_start(out=wqk_sb[:], in_=wqk.rearrange("(cb p) j -> p cb j", p=P))
    nc.sync.dma_start(out=wv_sb[:], in_=wv.rearrange("(cb p) j -> p cb j", p=P))
    nc.sync.dma_start(out=wp_sb[:], in_=wp.rearrange("(rb p) j -> p rb j", p=P))

    # ---- causal mask [128, 896]: maskf[p, g] = 1.0 if g >= p + 384 else 0.0
    # For a scoresT tile at offset r = 128*t2 - 512*t1c (r in {0,128,256,384}),
    # slice maskf[:, 384-r : 896-r] gives keep(p, f) = (f >= p + r).
    nc.gpsimd.memset(maskf[:], 1.0)
    nc.gpsimd.affine_select(
        out=maskf[:],
        in_=maskf[:],
        compare_op=mybir.AluOpType.is_ge,
        fill=0.0,
        base=-384,
        channel_multiplier=-1,
        pattern=[[1, 896]],
    )
    # ones column for the denominator row of the AV matmul
    nc.gpsimd.memset(v_aug[:, :, :, D], 1.0)

    # ---- qkv projections ----
    # q,k (column-parallel): psum[j, t] = sum_c wqk[c, j] * xT[c, t]
    for jb in range(4):
        dst = (qT[0], qT[1], kT[0], kT[1])[jb]
        for t1c in range(NT1):
            ps = psum.tile([P, TCH], F32, tag="mm", bufs=2, name=f"qk_ps_{jb}_{t1c}")
            for cb in range(NCB):
                nc.tensor.matmul(
                    ps[:],
                    lhsT=wqk_sb[:, cb, jb * P:(jb + 1) * P],
                    rhs=xT[cb][:, t1c * TCH:(t1c + 1) * TCH],
                    start=(cb == 0),
                    stop=(cb == NCB - 1),
                )
            nc.any.tensor_copy(out=dst[:, t1c * TCH:(t1c + 1) * TCH], in_=ps[:])
    # v (natural layout): psum[t, j] = sum_c xT[c, t] * wv[c, j]
    for tt in range(NT2):
        ps = psum.tile([P, TCH], F32, tag="mm", bufs=2, name=f"v_ps_{tt}")
        for cb in range(NCB):
            nc.tensor.matmul(
                ps[:, 0:256],
                lhsT=xT[cb][:, tt * P:(tt + 1) * P],
                rhs=wv_sb[:, cb, :],
                start=(cb == 0),
                stop=(cb == NCB - 1),
            )
        nc.any.tensor_copy(
            out=v_aug[:, tt, :, 0:D],
            in_=ps[:, 0:256].rearrange("p (h d) -> p h d", h=HEADS_PER_CORE),
        )

    # ---- attention, software-pipelined by one (t1c, head-pair) group ----
    # group = (t1c, hp): scores+exp for both heads of pair hp at t1 chunk t1c,
    # then (one group later) the AV matmuls + normalize.
    groups = [(3, 0), (0, 1), (2, 0), (1, 1), (1, 0), (2, 1), (0, 0), (3, 1)]
    attn_tiles = {}

    def emit_scores(t1c, hp):
        n_t2 = 4 * (t1c + 1)
        for t2 in range(n_t2):
            # columns [0, r) of this scoresT tile are fully masked by causality;
            # compute/exp only the valid range [r, TCH)
            r = max(t2 * P - t1c * TCH, 0)
            w = TCH - r
            for h in range(2):
                sc = psum.tile([P, TCH], F32, tag="sc", bufs=4,
                               name=f"sc_{t1c}_{hp}_{t2}_{h}")
                nc.tensor.matmul(
                    sc[:, 0:w],
                    lhsT=kT[hp][64 * h:64 * h + 64, t2 * P:(t2 + 1) * P],
                    rhs=qT[hp][64 * h:64 * h + 64, t1c * TCH + r:(t1c + 1) * TCH],
                    start=True,
                    stop=True,
                )
                at = attn_pool.tile([P, TCH], BF16, tag="attn", bufs=44,
                                    name=f"at_{t1c}_{hp}_{t2}_{h}")
                if r > 0:
                    nc.vector.memset(at[:, 0:r], 0.0)
                nc.scalar.activation(
                    out=at[:, r:TCH], in_=sc[:, 0:w],
                    func=mybir.ActivationFunctionType.Exp,
                    scale=0.125,
                )
                if t2 * P - t1c * TCH >= 0:
                    # in-tile triangle: keep (p, f') iff f' >= p
                    nc.vector.tensor_mul(at[:, r:TCH], at[:, r:TCH],
                                         maskf[:, 384:384 + w])
                attn_tiles[(t1c, hp, t2, h)] = at

    def emit_av(t1c, hp):
        n_t2 = 4 * (t1c + 1)
        for h in range(2):
            yps = psum.tile([P, TCH], F32, tag="av", bufs=2,
                            name=f"yps_{t1c}_{hp}_{h}")
            for t2 in range(n_t2):
                nc.tensor.matmul(
                    yps[0:D + 1, :],
                    lhsT=v_aug[:, t2, 2 * hp + h, :],
                    rhs=attn_tiles.pop((t1c, hp, t2, h))[:],
                    start=(t2 == 0),
                    stop=(t2 == n_t2 - 1),
                )
            rc = work.tile([1, TCH], F32, tag="recip", bufs=2,
                           name=f"rc_{t1c}_{hp}_{h}")
            bc = work.tile([P, TCH], F32, tag="bc", bufs=2,
                           name=f"bc_{t1c}_{hp}_{h}")
            nc.vector.reciprocal(rc[:], yps[D:D + 1, :])
            nc.gpsimd.partition_broadcast(bc[0:D, :], rc[:])
            nc.vector.tensor_mul(
                yT[hp][64 * h:64 * h + 64, t1c * TCH:(t1c + 1) * TCH],
                yps[0:D, :],
                bc[0:D, :],
            )

    emit_scores(*groups[0])
    for i in range(1, len(groups)):
        emit_scores(*groups[i])
        emit_av(*groups[i - 1])
    emit_av(*groups[-1])

    # ---- output projection (row-parallel partial) ----
    # psum[t, oc] = sum_ch yT[ch, t] * wp[ch, oc], ch split over the 2 pairs
    for tt in range(NT2):
        for ocb in range(2):
            ps = psum.tile([P, TCH], F32, tag="mm", bufs=2,
                           name=f"pr_ps_{tt}_{ocb}")
            for hp in range(2):
                nc.tensor.matmul(
                    ps[:],
                    lhsT=yT[hp][:, tt * P:(tt + 1) * P],
                    rhs=wp_sb[:, hp, ocb * TCH:(ocb + 1) * TCH],
                    start=(hp == 0),
                    stop=(hp == 1),
                )
            ot = work.tile([P, TCH], F32, tag="osb", bufs=3, name=f"ot_{tt}_{ocb}")
            nc.vector.tensor_copy(out=ot[:], in_=ps[:])
            nc.sync.dma_start(
                out=out[tt * P:(tt + 1) * P, ocb * TCH:(ocb + 1) * TCH],
                in_=ot[:],
            )
    ctx.close()


def shard_inputs(x, w_attn, w_proj):
    """Full inputs -> per-core in_maps (8 cores: batch x head-group)."""
    bf = ml_dtypes.bfloat16
    in_maps = []
    for core in range(N_CORES):
        b, hg = divmod(core, HEADS_PER_CORE)
        heads = [hg * HEADS_PER_CORE + j for j in range(HEADS_PER_CORE)]
        wqk = np.concatenate(
            [w_attn[:, 64 * h:64 * h + 64] for h in heads]
            + [w_attn[:, C + 64 * h:C + 64 * h + 64] for h in heads], axis=1)
        wv = np.concatenate(
            [w_attn[:, 2 * C + 64 * h:2 * C + 64 * h + 64] for h in heads], axis=1)
        wp = np.concatenate([w_proj[64 * h:64 * h + 64, :] for h in heads], axis=0)
        in_maps.append({
            "x": np.ascontiguousarray(x[b].T).astype(bf),
            "wqk": np.ascontiguousarray(wqk).astype(bf),
            "wv": np.ascontiguousarray(wv).astype(bf),
            "wp": np.ascontiguousarray(wp).astype(bf),
        })
    return in_maps


_NC_CACHE = None


def _get_program():
    global _NC_CACHE
    if _NC_CACHE is None:
        _NC_CACHE = build_program()
    return _NC_CACHE


def kernel(x, w_attn, w_proj, _run_kwargs=None):
    x = np.asarray(x, dtype=np.float32)
    w_attn = np.asarray(w_attn, dtype=np.float32)
    w_proj = np.asarray(w_proj, dtype=np.float32)
    nc = _get_program()
    in_maps = shard_inputs(x, w_attn, w_proj)
    res = run_bass_kernel_spmd(nc, in_maps, core_ids=list(range(N_CORES)),
                               **(_run_kwargs or {}))
    y = np.zeros((B, T, C), dtype=np.float32)
    for core in range(N_CORES):
        b = core // HEADS_PER_CORE
        y[b] += res.results[core]["out"]
    if _run_kwargs:
        return y, res
    return y


# revision 17
# speedup vs baseline: 1.4189x; 1.4189x over previous
"""Causal self-attention (B=2, T=2048, C=1024, H=16, D=64) on 8 TRN2 NeuronCores.

Sharding: batch x head-group. Core i handles batch b=i//4 and heads
[4*(i%4) .. 4*(i%4)+3]. c_attn is column-parallel (each core computes q,k,v
only for its 4 heads), c_proj is row-parallel (each core produces a partial
[T, C] output; the all-reduce over the 4 cores of a batch happens on the host
during unsharding).

Per-core kernel layout (all matmul inputs bf16, PSUM accumulation fp32):
  xT[cb]  [128, 2048]  x^T per 128-wide channel block (DMA xbar transpose)
  qT/kT   [128, 2048]  per head-pair (h0 rows 0:64, h1 rows 64:128)
  v_aug   [128, 16, 4, 65]  v tiles [t2-tile, head] with a ones column (row
                            sums of attn emerge as row 64 of the AV psum)
  scoresT = kT.T-tile @ qT-chunk -> [128 t2, 512 t1] psum, exp on ACT,
  causal mask as 0/1 multiply on DVE (diagonal tiles only),
  yT_aug = v_aug.T @ attnT accumulated over t2, normalize by row 64,
  out_partial = yT.T-tile @ w_proj rows.
"""

import sys

sys.path.insert(0, "/opt/trn_rl_repo")

import numpy as np
import ml_dtypes

import concourse.bacc as bacc
import concourse.mybir as mybir
import concourse.tile as tile
from concourse.bass_utils import run_bass_kernel_spmd

BF16 = mybir.dt.bfloat16
F32 = mybir.dt.float32

B, T, C = 2, 2048, 1024
H, D = 16, 64
HEADS_PER_CORE = 4
N_CORES = 8
P = 128
TCH = 512            # t1 moving-chunk width
NT1 = T // TCH       # 4 t1 chunks
NT2 = T // P         # 16 t2 tiles
NCB = C // P         # 8 channel blocks


def build_program():
    nc = bacc.Bacc("TRN2", target_bir_lowering=False, debug=False)
    # x arrives pre-transposed from the host: x[c, t]
    x = nc.declare_dram_parameter("x", [C, T], BF16, isOutput=False)
    wqk = nc.declare_dram_parameter("wqk", [C, 512], BF16, isOutput=False)
    wv = nc.declare_dram_parameter("wv", [C, 256], BF16, isOutput=False)
    wp = nc.declare_dram_parameter("wp", [256, C], BF16, isOutput=False)
    out = nc.declare_dram_parameter("out", [T, C], F32, isOutput=True)

    with tile.TileContext(nc) as tc:
        build_body(nc, tc, x, wqk, wv, wp, out)
    nc.compile()
    return nc


def build_body(nc, tc, x, wqk, wv, wp, out):
    from contextlib import ExitStack

    ctx = ExitStack()
    pers = ctx.enter_context(tc.tile_pool(name="pers", bufs=1))
    work = ctx.enter_context(tc.tile_pool(name="work", bufs=2))
    attn_pool = ctx.enter_context(tc.tile_pool(name="attn", bufs=44))
    psum = ctx.enter_context(tc.tile_pool(name="psum", bufs=2, space="PSUM"))

    # ---- persistent SBUF tensors ----
    xT = [pers.tile([P, T], BF16, tag=f"xT{cb}", name=f"xT{cb}") for cb in range(NCB)]
    wqk_sb = pers.tile([P, NCB, 512], BF16, tag="wqk", name="wqk_sb")
    wv_sb = pers.tile([P, NCB, 256], BF16, tag="wv", name="wv_sb")
    wp_sb = pers.tile([P, 2, C], BF16, tag="wp", name="wp_sb")
    # q transposed, head pairs stacked on partitions; k transposed per head,
    # zero-padded to K=128 so the scores matmuls run in 128x128 tile mode
    # (mixing 64-row and 128-row modes forces a PE drain per mode switch)
    qT = [pers.tile([P, T], BF16, tag=f"qT{i}", name=f"qT{i}") for i in range(2)]
    kTh = [pers.tile([P, T], BF16, tag=f"kTh{i}", name=f"kTh{i}") for i in range(4)]
    v_aug = pers.tile([P, NT2, HEADS_PER_CORE, D + 1], BF16, tag="vaug", name="v_aug")
    yT = [pers.tile([P, T], BF16, tag=f"yT{i}", name=f"yT{i}") for i in range(2)]

    # ---- input DMAs (weights first; alternate HWDGE engines) ----
    nc.sync.dma_start(out=wqk_sb[:], in_=wqk.rearrange("(cb p) j -> p cb j", p=P))
    nc.scalar.dma_start(out=wv_sb[:], in_=wv.rearrange("(cb p) j -> p cb j", p=P))
    for cb in range(NCB):
        eng = nc.sync if cb % 2 == 0 else nc.scalar
        eng.dma_start(out=xT[cb][:], in_=x[cb * P:(cb + 1) * P, :])
    nc.scalar.dma_start(out=wp_sb[:], in_=wp.rearrange("(rb p) j -> p rb j", p=P))

    # ones column for the denominator row of the AV matmul
    nc.gpsimd.memset(v_aug[:, :, :, D], 1.0)
    # zero-fill per-head kT (evacs only write the head's own 64 rows)
    for i in range(4):
        nc.vector.memset(kTh[i][:], 0.0)

    # ---- qkv projections ----
    # q,k (column-parallel): psum[j, t] = sum_c wqk[c, j] * xT[c, t]
    for jb in range(4):
        for t1c in range(NT1):
            ps = psum.tile([P, TCH], F32, tag="mm", bufs=2, name=f"qk_ps_{jb}_{t1c}")
            for cb in range(NCB):
                nc.tensor.matmul(
                    ps[:],
                    lhsT=wqk_sb[:, cb, jb * P:(jb + 1) * P],
                    rhs=xT[cb][:, t1c * TCH:(t1c + 1) * TCH],
                    start=(cb == 0),
                    stop=(cb == NCB - 1),
                )
            sl = slice(t1c * TCH, (t1c + 1) * TCH)
            if jb < 2:
                nc.any.tensor_copy(out=qT[jb][:, sl], in_=ps[:])
            else:
                hp = jb - 2
                nc.any.tensor_copy(out=kTh[2 * hp][0:64, sl], in_=ps[0:64, :])
                nc.any.tensor_copy(out=kTh[2 * hp + 1][64:P, sl], in_=ps[64:P, :])
    # v (natural layout): psum[t, j] = sum_c xT[c, t] * wv[c, j]
    for tt in range(NT2):
        ps = psum.tile([P, TCH], F32, tag="mm", bufs=2, name=f"v_ps_{tt}")
        for cb in range(NCB):
            nc.tensor.matmul(
                ps[:, 0:256],
                lhsT=xT[cb][:, tt * P:(tt + 1) * P],
                rhs=wv_sb[:, cb, :],
                start=(cb == 0),
                stop=(cb == NCB - 1),
            )
        nc.any.tensor_copy(
            out=v_aug[:, tt, :, 0:D],
            in_=ps[:, 0:256].rearrange("p (h d) -> p h d", h=HEADS_PER_CORE),
        )

    # ---- attention, software-pipelined by one (t1c, head-pair) group ----
    # group = (t1c, hp): scores+exp for both heads of pair hp at t1 chunk t1c,
    # then (one group later) the AV matmuls + normalize.
    groups = [(3, 0), (0, 1), (2, 0), (1, 1), (1, 0), (2, 1), (0, 0), (3, 1)]
    attn_tiles = {}

    def emit_scores(t1c, hp):
        n_t2 = 4 * (t1c + 1)
        for t2 in range(n_t2):
            # columns [0, r) of this scoresT tile are fully masked by causality;
            # compute/exp only the valid range [r, TCH)
            r = max(t2 * P - t1c * TCH, 0)
            w = TCH - r
            for h in range(2):
                sc = psum.tile([P, TCH], F32, tag="sc", bufs=4,
                               name=f"sc_{t1c}_{hp}_{t2}_{h}")
                nc.tensor.matmul(
                    sc[:, 0:w],
                    lhsT=kTh[2 * hp + h][:, t2 * P:(t2 + 1) * P],
                    rhs=qT[hp][:, t1c * TCH + r:(t1c + 1) * TCH],
                    start=True,
                    stop=True,
                )
                at = attn_pool.tile([P, TCH], BF16, tag="attn", bufs=44,
                                    name=f"at_{t1c}_{hp}_{t2}_{h}")
                nc.scalar.activation(
                    out=at[:, r:TCH], in_=sc[:, 0:w],
                    func=mybir.ActivationFunctionType.Exp,
                    scale=0.125,
                )
                if t2 * P - t1c * TCH >= 0:
                    # in-tile triangle: keep (p, f') iff f' >= p, zero the rest
                    nc.gpsimd.affine_select(
                        out=at[:, r:TCH], in_=at[:, r:TCH],
                        compare_op=mybir.AluOpType.is_ge,
                        fill=0.0,
                        base=0,
                        channel_multiplier=-1,
                        pattern=[[1, w]],
                    )
                attn_tiles[(t1c, hp, t2, h)] = at

    def emit_av(t1c, hp):
        n_t2 = 4 * (t1c + 1)
        for h in range(2):
            yps = psum.tile([P, TCH], F32, tag="av", bufs=2,
                            name=f"yps_{t1c}_{hp}_{h}")
            for t2 in range(n_t2):
                # columns [0, r) of the attn tile are causally zero -> skip
                r = max(t2 * P - t1c * TCH, 0)
                nc.tensor.matmul(
                    yps[0:D + 1, r:TCH],
                    lhsT=v_aug[:, t2, 2 * hp + h, :],
                    rhs=attn_tiles.pop((t1c, hp, t2, h))[:, r:TCH],
                    start=(t2 == 0),
                    stop=(t2 == n_t2 - 1),
                )
            rc = work.tile([1, TCH], F32, tag="rc", bufs=2,
                           name=f"rc_{t1c}_{hp}_{h}")
            bc = work.tile([P, TCH], F32, tag="bc", bufs=2,
                           name=f"bc_{t1c}_{hp}_{h}")
            nc.vector.reciprocal(rc[:], yps[D:D + 1, :])
            nc.gpsimd.partition_broadcast(bc[0:D, :], rc[:])
            nc.vector.tensor_mul(
                yT[hp][64 * h:64 * h + 64, t1c * TCH:(t1c + 1) * TCH],
                yps[0:D, :],
                bc[0:D, :],
            )

    emit_scores(*groups[0])
    for i in range(1, len(groups)):
        emit_scores(*groups[i])
        emit_av(*groups[i - 1])
    emit_av(*groups[-1])

    # ---- output projection (row-parallel partial) ----
    # psum[t, oc] = sum_ch yT[ch, t] * wp[ch, oc], ch split over the 2 pairs
    for tt in range(NT2):
        for ocb in range(2):
            ps = psum.tile([P, TCH], F32, tag="mm", bufs=2,
                           name=f"pr_ps_{tt}_{ocb}")
            for hp in range(2):
                nc.tensor.matmul(
                    ps[:],
                    lhsT=yT[hp][:, tt * P:(tt + 1) * P],
                    rhs=wp_sb[:, hp, ocb * TCH:(ocb + 1) * TCH],
                    start=(hp == 0),
                    stop=(hp == 1),
                )
            ot = work.tile([P, TCH], F32, tag="osb", bufs=3, name=f"ot_{tt}_{ocb}")
            nc.any.tensor_copy(out=ot[:], in_=ps[:])
            nc.sync.dma_start(
                out=out[tt * P:(tt + 1) * P, ocb * TCH:(ocb + 1) * TCH],
                in_=ot[:],
            )
    ctx.close()


def shard_inputs(x, w_attn, w_proj):
    """Full inputs -> per-core in_maps (8 cores: batch x head-group)."""
    bf = ml_dtypes.bfloat16
    in_maps = []
    for core in range(N_CORES):
        b, hg = divmod(core, HEADS_PER_CORE)
        heads = [hg * HEADS_PER_CORE + j for j in range(HEADS_PER_CORE)]
        wqk = np.concatenate(
            [w_attn[:, 64 * h:64 * h + 64] for h in heads]
            + [w_attn[:, C + 64 * h:C + 64 * h + 64] for h in heads], axis=1)
        wv = np.concatenate(
            [w_attn[:, 2 * C + 64 * h:2 * C + 64 * h + 64] for h in heads], axis=1)
        wp = np.concatenate([w_proj[64 * h:64 * h + 64, :] for h in heads], axis=0)
        in_maps.append({
            "x": np.ascontiguousarray(x[b].T).astype(bf),
            "wqk": np.ascontiguousarray(wqk).astype(bf),
            "wv": np.ascontiguousarray(wv).astype(bf),
            "wp": np.ascontiguousarray(wp).astype(bf),
        })
    return in_maps


_NC_CACHE = None


def _get_program():
    global _NC_CACHE
    if _NC_CACHE is None:
        _NC_CACHE = build_program()
    return _NC_CACHE


def kernel(x, w_attn, w_proj, _run_kwargs=None):
    x = np.asarray(x, dtype=np.float32)
    w_attn = np.asarray(w_attn, dtype=np.float32)
    w_proj = np.asarray(w_proj, dtype=np.float32)
    nc = _get_program()
    in_maps = shard_inputs(x, w_attn, w_proj)
    res = run_bass_kernel_spmd(nc, in_maps, core_ids=list(range(N_CORES)),
                               **(_run_kwargs or {}))
    y = np.zeros((B, T, C), dtype=np.float32)
    for core in range(N_CORES):
        b = core // HEADS_PER_CORE
        y[b] += res.results[core]["out"]
    if _run_kwargs:
        return y, res
    return y


# revision 20
# speedup vs baseline: 1.4849x; 1.0465x over previous
"""Causal self-attention (B=2, T=2048, C=1024, H=16, D=64) on 8 TRN2 NeuronCores.

Sharding: batch x head-group. Core i handles batch b=i//4 and heads
[4*(i%4) .. 4*(i%4)+3]. c_attn is column-parallel (each core computes q,k,v
only for its 4 heads), c_proj is row-parallel (each core produces a partial
[T, C] output; the all-reduce over the 4 cores of a batch happens on the host
during unsharding).

Per-core kernel layout (all matmul inputs bf16, PSUM accumulation fp32):
  xT[cb]  [128, 2048]  x^T per 128-wide channel block (DMA xbar transpose)
  qT/kT   [128, 2048]  per head-pair (h0 rows 0:64, h1 rows 64:128)
  v_aug   [128, 16, 4, 65]  v tiles [t2-tile, head] with a ones column (row
                            sums of attn emerge as row 64 of the AV psum)
  scoresT = kT.T-tile @ qT-chunk -> [128 t2, 512 t1] psum, exp on ACT,
  causal mask as 0/1 multiply on DVE (diagonal tiles only),
  yT_aug = v_aug.T @ attnT accumulated over t2, normalize by row 64,
  out_partial = yT.T-tile @ w_proj rows.
"""

import sys

sys.path.insert(0, "/opt/trn_rl_repo")

import numpy as np
import ml_dtypes

import concourse.bacc as bacc
import concourse.mybir as mybir
import concourse.tile as tile
from concourse.bass_utils import run_bass_kernel_spmd

BF16 = mybir.dt.bfloat16
F32 = mybir.dt.float32

B, T, C = 2, 2048, 1024
H, D = 16, 64
HEADS_PER_CORE = 4
N_CORES = 8
P = 128
TCH = 512            # t1 moving-chunk width
NT1 = T // TCH       # 4 t1 chunks
NT2 = T // P         # 16 t2 tiles
NCB = C // P         # 8 channel blocks


def build_program():
    nc = bacc.Bacc("TRN2", target_bir_lowering=False, debug=False)
    # x arrives pre-transposed from the host: x[c, t]
    x = nc.declare_dram_parameter("x", [C, T], BF16, isOutput=False)
    wqk = nc.declare_dram_parameter("wqk", [C, 512], BF16, isOutput=False)
    wv = nc.declare_dram_parameter("wv", [C, 256], BF16, isOutput=False)
    wp = nc.declare_dram_parameter("wp", [256, C], BF16, isOutput=False)
    out = nc.declare_dram_parameter("out", [T, C], F32, isOutput=True)

    with tile.TileContext(nc) as tc:
        build_body(nc, tc, x, wqk, wv, wp, out)
    nc.compile()
    return nc


def build_body(nc, tc, x, wqk, wv, wp, out):
    from contextlib import ExitStack

    ctx = ExitStack()
    pers = ctx.enter_context(tc.tile_pool(name="pers", bufs=1))
    work = ctx.enter_context(tc.tile_pool(name="work", bufs=2))
    attn_pool = ctx.enter_context(tc.tile_pool(name="attn", bufs=44))
    psum = ctx.enter_context(tc.tile_pool(name="psum", bufs=2, space="PSUM"))

    # ---- persistent SBUF tensors ----
    xT = [pers.tile([P, T], BF16, tag=f"xT{cb}", name=f"xT{cb}") for cb in range(NCB)]
    wqk_sb = pers.tile([P, NCB, 512], BF16, tag="wqk", name="wqk_sb")
    wv_sb = pers.tile([P, NCB, 256], BF16, tag="wv", name="wv_sb")
    wp_sb = pers.tile([P, 2, C], BF16, tag="wp", name="wp_sb")
    # q transposed, head pairs stacked on partitions; k transposed per head,
    # zero-padded to K=128 so the scores matmuls run in 128x128 tile mode
    # (mixing 64-row and 128-row modes forces a PE drain per mode switch)
    qT = [pers.tile([P, T], BF16, tag=f"qT{i}", name=f"qT{i}") for i in range(2)]
    kTh = [pers.tile([P, T], BF16, tag=f"kTh{i}", name=f"kTh{i}") for i in range(4)]
    v_aug = pers.tile([P, NT2, HEADS_PER_CORE, D + 1], BF16, tag="vaug", name="v_aug")
    yT = [pers.tile([P, T], BF16, tag=f"yT{i}", name=f"yT{i}") for i in range(2)]

    # ---- input DMAs (weights first; alternate HWDGE engines) ----
    nc.sync.dma_start(out=wqk_sb[:], in_=wqk.rearrange("(cb p) j -> p cb j", p=P))
    nc.scalar.dma_start(out=wv_sb[:], in_=wv.rearrange("(cb p) j -> p cb j", p=P))
    for cb in range(NCB):
        eng = nc.sync if cb % 2 == 0 else nc.scalar
        eng.dma_start(out=xT[cb][:], in_=x[cb * P:(cb + 1) * P, :])
    nc.scalar.dma_start(out=wp_sb[:], in_=wp.rearrange("(rb p) j -> p rb j", p=P))

    # ones column for the denominator row of the AV matmul
    nc.gpsimd.memset(v_aug[:, :, :, D], 1.0)
    # zero-fill per-head kT (evacs only write the head's own 64 rows)
    for i in range(4):
        nc.vector.memset(kTh[i][:], 0.0)

    # ---- qkv projections ----
    # q,k (column-parallel): psum[j, t] = sum_c wqk[c, j] * xT[c, t]
    for jb in range(4):
        for t1c in range(NT1):
            ps = psum.tile([P, TCH], F32, tag="mm", bufs=2, name=f"qk_ps_{jb}_{t1c}")
            for cb in range(NCB):
                nc.tensor.matmul(
                    ps[:],
                    lhsT=wqk_sb[:, cb, jb * P:(jb + 1) * P],
                    rhs=xT[cb][:, t1c * TCH:(t1c + 1) * TCH],
                    start=(cb == 0),
                    stop=(cb == NCB - 1),
                )
            sl = slice(t1c * TCH, (t1c + 1) * TCH)
            if jb < 2:
                nc.vector.tensor_copy(out=qT[jb][:, sl], in_=ps[:])
            else:
                hp = jb - 2
                nc.vector.tensor_copy(out=kTh[2 * hp][0:64, sl], in_=ps[0:64, :])
                nc.vector.tensor_copy(out=kTh[2 * hp + 1][64:P, sl], in_=ps[64:P, :])
    # v (natural layout): psum[t, j] = sum_c xT[c, t] * wv[c, j]
    for tt in range(NT2):
        ps = psum.tile([P, TCH], F32, tag="mm", bufs=2, name=f"v_ps_{tt}")
        for cb in range(NCB):
            nc.tensor.matmul(
                ps[:, 0:256],
                lhsT=xT[cb][:, tt * P:(tt + 1) * P],
                rhs=wv_sb[:, cb, :],
                start=(cb == 0),
                stop=(cb == NCB - 1),
            )
        nc.vector.tensor_copy(
            out=v_aug[:, tt, :, 0:D],
            in_=ps[:, 0:256].rearrange("p (h d) -> p h d", h=HEADS_PER_CORE),
        )

    # ---- attention, software-pipelined by one (t1c, head-pair) group ----
    # group = (t1c, hp): scores+exp for both heads of pair hp at t1 chunk t1c,
    # then (one group later) the AV matmuls + normalize.
    groups = [(3, 0), (0, 1), (2, 0), (1, 1), (1, 0), (2, 1), (0, 0), (3, 1)]
    attn_tiles = {}

    def emit_scores(t1c, hp):
        n_t2 = 4 * (t1c + 1)
        for t2 in range(n_t2):
            # columns [0, r) of this scoresT tile are fully masked by causality;
            # compute/exp only the valid range [r, TCH)
            r = max(t2 * P - t1c * TCH, 0)
            w = TCH - r
            for h in range(2):
                sc = psum.tile([P, TCH], F32, tag="sc", bufs=4,
                               name=f"sc_{t1c}_{hp}_{t2}_{h}")
                nc.tensor.matmul(
                    sc[:, 0:w],
                    lhsT=kTh[2 * hp + h][:, t2 * P:(t2 + 1) * P],
                    rhs=qT[hp][:, t1c * TCH + r:(t1c + 1) * TCH],
                    start=True,
                    stop=True,
                )
                at = attn_pool.tile([P, TCH], BF16, tag="attn", bufs=44,
                                    name=f"at_{t1c}_{hp}_{t2}_{h}")
                nc.scalar.activation(
                    out=at[:, r:TCH], in_=sc[:, 0:w],
                    func=mybir.ActivationFunctionType.Exp,
                    scale=0.125,
                )
                if t2 * P - t1c * TCH >= 0:
                    # in-tile triangle: keep (p, f') iff f' >= p, zero the rest
                    nc.gpsimd.affine_select(
                        out=at[:, r:TCH], in_=at[:, r:TCH],
                        compare_op=mybir.AluOpType.is_ge,
                        fill=0.0,
                        base=0,
                        channel_multiplier=-1,
                        pattern=[[1, w]],
                    )
                attn_tiles[(t1c, hp, t2, h)] = at

    def emit_av(t1c, hp):
        n_t2 = 4 * (t1c + 1)
        for h in range(2):
            yps = psum.tile([P, TCH], F32, tag="av", bufs=2,
                            name=f"yps_{t1c}_{hp}_{h}")
            for t2 in range(n_t2):
                # columns [0, r) of the attn tile are causally zero -> skip
                r = max(t2 * P - t1c * TCH, 0)
                nc.tensor.matmul(
                    yps[0:D + 1, r:TCH],
                    lhsT=v_aug[:, t2, 2 * hp + h, :],
                    rhs=attn_tiles.pop((t1c, hp, t2, h))[:, r:TCH],
                    start=(t2 == 0),
                    stop=(t2 == n_t2 - 1),
                )
            # stage to SBUF immediately so the PSUM bank frees for the next
            # group; the reciprocal/normalize chain then runs off the PE's
            # critical path
            ys = work.tile([D + 1, TCH], F32, tag="ys", bufs=4,
                           name=f"ys_{t1c}_{hp}_{h}")
            nc.any.tensor_copy(out=ys[:], in_=yps[0:D + 1, :])
            rc = work.tile([1, TCH], F32, tag="rc", bufs=4,
                           name=f"rc_{t1c}_{hp}_{h}")
            bc = work.tile([P, TCH], F32, tag="bc", bufs=4,
                           name=f"bc_{t1c}_{hp}_{h}")
            nc.vector.reciprocal(rc[:], ys[D:D + 1, :])
            nc.gpsimd.partition_broadcast(bc[0:D, :], rc[:])
            nc.vector.tensor_mul(
                yT[hp][64 * h:64 * h + 64, t1c * TCH:(t1c + 1) * TCH],
                ys[0:D, :],
                bc[0:D, :],
            )

    emit_scores(*groups[0])
    for i in range(1, len(groups)):
        emit_scores(*groups[i])
        emit_av(*groups[i - 1])
    emit_av(*groups[-1])

    # ---- output projection (row-parallel partial) ----
    # psum[t, oc] = sum_ch yT[ch, t] * wp[ch, oc], ch split over the 2 pairs
    for tt in range(NT2):
        for ocb in range(2):
            ps = psum.tile([P, TCH], F32, tag="mm", bufs=2,
                           name=f"pr_ps_{tt}_{ocb}")
            for hp in range(2):
                nc.tensor.matmul(
                    ps[:],
                    lhsT=yT[hp][:, tt * P:(tt + 1) * P],
                    rhs=wp_sb[:, hp, ocb * TCH:(ocb + 1) * TCH],
                    start=(hp == 0),
                    stop=(hp == 1),
                )
            ot = work.tile([P, TCH], F32, tag="osb", bufs=3, name=f"ot_{tt}_{ocb}")
            nc.any.tensor_copy(out=ot[:], in_=ps[:])
            nc.sync.dma_start(
                out=out[tt * P:(tt + 1) * P, ocb * TCH:(ocb + 1) * TCH],
                in_=ot[:],
            )
    ctx.close()


def shard_inputs(x, w_attn, w_proj):
    """Full inputs -> per-core in_maps (8 cores: batch x head-group)."""
    bf = ml_dtypes.bfloat16
    in_maps = []
    for core in range(N_CORES):
        b, hg = divmod(core, HEADS_PER_CORE)
        heads = [hg * HEADS_PER_CORE + j for j in range(HEADS_PER_CORE)]
        wqk = np.concatenate(
            [w_attn[:, 64 * h:64 * h + 64] for h in heads]
            + [w_attn[:, C + 64 * h:C + 64 * h + 64] for h in heads], axis=1)
        wv = np.concatenate(
            [w_attn[:, 2 * C + 64 * h:2 * C + 64 * h + 64] for h in heads], axis=1)
        wp = np.concatenate([w_proj[64 * h:64 * h + 64, :] for h in heads], axis=0)
        in_maps.append({
            "x": np.ascontiguousarray(x[b].T).astype(bf),
            "wqk": np.ascontiguousarray(wqk).astype(bf),
            "wv": np.ascontiguousarray(wv).astype(bf),
            "wp": np.ascontiguousarray(wp).astype(bf),
        })
    return in_maps


_NC_CACHE = None


def _get_program():
    global _NC_CACHE
    if _NC_CACHE is None:
        _NC_CACHE = build_program()
    return _NC_CACHE


def kernel(x, w_attn, w_proj, _run_kwargs=None):
    x = np.asarray(x, dtype=np.float32)
    w_attn = np.asarray(w_attn, dtype=np.float32)
    w_proj = np.asarray(w_proj, dtype=np.float32)
    nc = _get_program()
    in_maps = shard_inputs(x, w_attn, w_proj)
    res = run_bass_kernel_spmd(nc, in_maps, core_ids=list(range(N_CORES)),
                               **(_run_kwargs or {}))
    y = np.zeros((B, T, C), dtype=np.float32)
    for core in range(N_CORES):
        b = core // HEADS_PER_CORE
        y[b] += res.results[core]["out"]
    if _run_kwargs:
        return y, res
    return y


# revision 21
# speedup vs baseline: 1.4946x; 1.0066x over previous
"""Causal self-attention (B=2, T=2048, C=1024, H=16, D=64) on 8 TRN2 NeuronCores.

Sharding: batch x head-group. Core i handles batch b=i//4 and heads
[4*(i%4) .. 4*(i%4)+3]. c_attn is column-parallel (each core computes q,k,v
only for its 4 heads), c_proj is row-parallel (each core produces a partial
[T, C] output; the all-reduce over the 4 cores of a batch happens on the host
during unsharding).

Per-core kernel layout (all matmul inputs bf16, PSUM accumulation fp32):
  xT[cb]  [128, 2048]  x^T per 128-wide channel block (DMA xbar transpose)
  qT/kT   [128, 2048]  per head-pair (h0 rows 0:64, h1 rows 64:128)
  v_aug   [128, 16, 4, 65]  v tiles [t2-tile, head] with a ones column (row
                            sums of attn emerge as row 64 of the AV psum)
  scoresT = kT.T-tile @ qT-chunk -> [128 t2, 512 t1] psum, exp on ACT,
  causal mask as 0/1 multiply on DVE (diagonal tiles only),
  yT_aug = v_aug.T @ attnT accumulated over t2, normalize by row 64,
  out_partial = yT.T-tile @ w_proj rows.
"""

import sys

sys.path.insert(0, "/opt/trn_rl_repo")

import numpy as np
import ml_dtypes

import concourse.bacc as bacc
import concourse.mybir as mybir
import concourse.tile as tile
from concourse.bass_utils import run_bass_kernel_spmd

BF16 = mybir.dt.bfloat16
F32 = mybir.dt.float32

B, T, C = 2, 2048, 1024
H, D = 16, 64
HEADS_PER_CORE = 4
N_CORES = 8
P = 128
TCH = 512            # t1 moving-chunk width
NT1 = T // TCH       # 4 t1 chunks
NT2 = T // P         # 16 t2 tiles
NCB = C // P         # 8 channel blocks


def build_program():
    nc = bacc.Bacc("TRN2", target_bir_lowering=False, debug=False)
    # x arrives pre-transposed from the host: x[c, t]
    x = nc.declare_dram_parameter("x", [C, T], BF16, isOutput=False)
    wqk = nc.declare_dram_parameter("wqk", [C, 512], BF16, isOutput=False)
    wv = nc.declare_dram_parameter("wv", [C, 256], BF16, isOutput=False)
    wp = nc.declare_dram_parameter("wp", [256, C], BF16, isOutput=False)
    out = nc.declare_dram_parameter("out", [T, C], F32, isOutput=True)

    with tile.TileContext(nc) as tc:
        build_body(nc, tc, x, wqk, wv, wp, out)
    nc.compile()
    return nc


def build_body(nc, tc, x, wqk, wv, wp, out):
    from contextlib import ExitStack

    ctx = ExitStack()
    pers = ctx.enter_context(tc.tile_pool(name="pers", bufs=1))
    work = ctx.enter_context(tc.tile_pool(name="work", bufs=2))
    attn_pool = ctx.enter_context(tc.tile_pool(name="attn", bufs=44))
    psum = ctx.enter_context(tc.tile_pool(name="psum", bufs=2, space="PSUM"))

    # ---- persistent SBUF tensors ----
    xT = [pers.tile([P, T], BF16, tag=f"xT{cb}", name=f"xT{cb}") for cb in range(NCB)]
    wqk_sb = pers.tile([P, NCB, 512], BF16, tag="wqk", name="wqk_sb")
    wv_sb = pers.tile([P, NCB, 256], BF16, tag="wv", name="wv_sb")
    wp_sb = pers.tile([P, 2, C], BF16, tag="wp", name="wp_sb")
    # q transposed, head pairs stacked on partitions; k transposed per head,
    # zero-padded to K=128 so the scores matmuls run in 128x128 tile mode
    # (mixing 64-row and 128-row modes forces a PE drain per mode switch)
    qT = [pers.tile([P, T], BF16, tag=f"qT{i}", name=f"qT{i}") for i in range(2)]
    kTh = [pers.tile([P, T], BF16, tag=f"kTh{i}", name=f"kTh{i}") for i in range(4)]
    v_aug = pers.tile([P, NT2, HEADS_PER_CORE, D + 1], BF16, tag="vaug", name="v_aug")
    yT = [pers.tile([P, T], BF16, tag=f"yT{i}", name=f"yT{i}") for i in range(2)]

    # ---- input DMAs (weights first; alternate HWDGE engines) ----
    nc.sync.dma_start(out=wqk_sb[:], in_=wqk.rearrange("(cb p) j -> p cb j", p=P))
    nc.scalar.dma_start(out=wv_sb[:], in_=wv.rearrange("(cb p) j -> p cb j", p=P))
    for cb in range(NCB):
        eng = nc.sync if cb % 2 == 0 else nc.scalar
        eng.dma_start(out=xT[cb][:], in_=x[cb * P:(cb + 1) * P, :])
    nc.scalar.dma_start(out=wp_sb[:], in_=wp.rearrange("(rb p) j -> p rb j", p=P))

    # ones column for the denominator row of the AV matmul
    nc.gpsimd.memset(v_aug[:, :, :, D], 1.0)
    # zero-fill per-head kT (evacs only write the head's own 64 rows)
    for i in range(4):
        nc.vector.memset(kTh[i][:], 0.0)

    # ---- qkv projections ----
    # q,k (column-parallel): psum[j, t] = sum_c wqk[c, j] * xT[c, t]
    for jb in range(4):
        for t1c in range(NT1):
            ps = psum.tile([P, TCH], F32, tag="mm", bufs=2, name=f"qk_ps_{jb}_{t1c}")
            for cb in range(NCB):
                nc.tensor.matmul(
                    ps[:],
                    lhsT=wqk_sb[:, cb, jb * P:(jb + 1) * P],
                    rhs=xT[cb][:, t1c * TCH:(t1c + 1) * TCH],
                    start=(cb == 0),
                    stop=(cb == NCB - 1),
                )
            sl = slice(t1c * TCH, (t1c + 1) * TCH)
            if jb < 2:
                nc.vector.tensor_copy(out=qT[jb][:, sl], in_=ps[:])
            else:
                hp = jb - 2
                nc.vector.tensor_copy(out=kTh[2 * hp][0:64, sl], in_=ps[0:64, :])
                nc.vector.tensor_copy(out=kTh[2 * hp + 1][64:P, sl], in_=ps[64:P, :])
    # v (natural layout): psum[t, j] = sum_c xT[c, t] * wv[c, j]
    for tt in range(NT2):
        ps = psum.tile([P, TCH], F32, tag="mm", bufs=2, name=f"v_ps_{tt}")
        for cb in range(NCB):
            nc.tensor.matmul(
                ps[:, 0:256],
                lhsT=xT[cb][:, tt * P:(tt + 1) * P],
                rhs=wv_sb[:, cb, :],
                start=(cb == 0),
                stop=(cb == NCB - 1),
            )
        nc.vector.tensor_copy(
            out=v_aug[:, tt, :, 0:D],
            in_=ps[:, 0:256].rearrange("p (h d) -> p h d", h=HEADS_PER_CORE),
        )

    # ---- attention, software-pipelined by one (t1c, head-pair) group with
    # fine-grained interleave: group g's AV matmuls are emitted between group
    # g+1's score units so the in-order PE never sits behind a long
    # scores-only or AV-only stretch (that alternation starved ACT/PE in
    # earlier versions). Both heads of a pair share one 2-bank scores psum
    # unit -> one exp and one mask select cover both heads, amortizing ACT's
    # per-instruction overhead.
    groups = [(3, 0), (0, 1), (2, 0), (1, 1), (1, 0), (2, 1), (0, 0), (3, 1)]
    attn_tiles = {}
    av_psum = {}

    def emit_score_unit(t1c, hp, t2):
        # columns [0, r) of this scoresT tile are fully masked by causality;
        # compute/exp only the valid range [r, TCH)
        r = max(t2 * P - t1c * TCH, 0)
        w = TCH - r
        sc = psum.tile([P, 2 * TCH], F32, tag="sc", bufs=2,
                       name=f"sc_{t1c}_{hp}_{t2}")
        for h in range(2):
            nc.tensor.matmul(
                sc[:, h * TCH:h * TCH + w],
                lhsT=kTh[2 * hp + h][:, t2 * P:(t2 + 1) * P],
                rhs=qT[hp][:, t1c * TCH + r:(t1c + 1) * TCH],
                start=True,
                stop=True,
            )
        at = attn_pool.tile([P, 2 * TCH], BF16, tag="attn", bufs=24,
                            name=f"at_{t1c}_{hp}_{t2}")
        sc_v = sc[:].rearrange("p (u f) -> p u f", u=2)[:, :, 0:w]
        at_v = at[:].rearrange("p (u f) -> p u f", u=2)[:, :, r:TCH]
        nc.scalar.activation(
            out=at_v, in_=sc_v,
            func=mybir.ActivationFunctionType.Exp,
            scale=0.125,
        )
        if t2 * P - t1c * TCH >= 0:
            # in-tile triangle (same for both heads): keep (p, f') iff f' >= p
            nc.gpsimd.affine_select(
                out=at_v, in_=at_v,
                compare_op=mybir.AluOpType.is_ge,
                fill=0.0,
                base=0,
                channel_multiplier=-1,
                pattern=[[0, 2], [1, w]],
            )
        attn_tiles[(t1c, hp, t2)] = at

    def emit_av_mm(t1c, hp, h, t2, n_t2):
        if (t1c, hp, h) not in av_psum:
            av_psum[(t1c, hp, h)] = psum.tile(
                [P, TCH], F32, tag="av", bufs=2, name=f"yps_{t1c}_{hp}_{h}")
        yps = av_psum[(t1c, hp, h)]
        # columns [0, r) of the attn tile are causally zero -> skip
        r = max(t2 * P - t1c * TCH, 0)
        at = attn_tiles[(t1c, hp, t2)]
        if h == 1:
            attn_tiles.pop((t1c, hp, t2))
        nc.tensor.matmul(
            yps[0:D + 1, r:TCH],
            lhsT=v_aug[:, t2, 2 * hp + h, :],
            rhs=at[:, h * TCH + r:(h + 1) * TCH],
            start=(t2 == 0),
            stop=(t2 == n_t2 - 1),
        )
        if t2 == n_t2 - 1:
            # stage to SBUF immediately so the PSUM bank frees; the
            # reciprocal/normalize chain runs off the PE's critical path
            del av_psum[(t1c, hp, h)]
            ys = work.tile([D + 1, TCH], F32, tag="ys", bufs=4,
                           name=f"ys_{t1c}_{hp}_{h}")
            nc.any.tensor_copy(out=ys[:], in_=yps[0:D + 1, :])
            rc = work.tile([1, TCH], F32, tag="rc", bufs=4,
                           name=f"rc_{t1c}_{hp}_{h}")
            bc = work.tile([P, TCH], F32, tag="bc", bufs=4,
                           name=f"bc_{t1c}_{hp}_{h}")
            nc.vector.reciprocal(rc[:], ys[D:D + 1, :])
            nc.gpsimd.partition_broadcast(bc[0:D, :], rc[:])
            nc.vector.tensor_mul(
                yT[hp][64 * h:64 * h + 64, t1c * TCH:(t1c + 1) * TCH],
                ys[0:D, :],
                bc[0:D, :],
            )

    def sc_jobs(t1c, hp):
        return [(t1c, hp, t2) for t2 in range(4 * (t1c + 1))]

    def av_jobs(t1c, hp):
        n_t2 = 4 * (t1c + 1)
        return [(t1c, hp, h, t2, n_t2) for h in range(2) for t2 in range(n_t2)]

    for scj in sc_jobs(*groups[0]):
        emit_score_unit(*scj)
    for i in range(1, len(groups)):
        scs = sc_jobs(*groups[i])
        avs = av_jobs(*groups[i - 1])
        k = 0
        for j, scj in enumerate(scs):
            emit_score_unit(*scj)
            while k < len(avs) and k * len(scs) < (j + 1) * len(avs):
                emit_av_mm(*avs[k])
                k += 1
        for job in avs[k:]:
            emit_av_mm(*job)
    for job in av_jobs(*groups[-1]):
        emit_av_mm(*job)

    # ---- output projection (row-parallel partial) ----
    # psum[t, oc] = sum_ch yT[ch, t] * wp[ch, oc], ch split over the 2 pairs
    for tt in range(NT2):
        for ocb in range(2):
            ps = psum.tile([P, TCH], F32, tag="mm", bufs=2,
                           name=f"pr_ps_{tt}_{ocb}")
            for hp in range(2):
                nc.tensor.matmul(
                    ps[:],
                    lhsT=yT[hp][:, tt * P:(tt + 1) * P],
                    rhs=wp_sb[:, hp, ocb * TCH:(ocb + 1) * TCH],
                    start=(hp == 0),
                    stop=(hp == 1),
                )
            ot = work.tile([P, TCH], F32, tag="osb", bufs=3, name=f"ot_{tt}_{ocb}")
            nc.any.tensor_copy(out=ot[:], in_=ps[:])
            nc.sync.dma_start(
                out=out[tt * P:(tt + 1) * P, ocb * TCH:(ocb + 1) * TCH],
                in_=ot[:],
            )
    ctx.close()


def shard_inputs(x, w_attn, w_proj):
    """Full inputs -> per-core in_maps (8 cores: batch x head-group)."""
    bf = ml_dtypes.bfloat16
    in_maps = []
    for core in range(N_CORES):
        b, hg = divmod(core, HEADS_PER_CORE)
        heads = [hg * HEADS_PER_CORE + j for j in range(HEADS_PER_CORE)]
        wqk = np.concatenate(
            [w_attn[:, 64 * h:64 * h + 64] for h in heads]
            + [w_attn[:, C + 64 * h:C + 64 * h + 64] for h in heads], axis=1)
        wv = np.concatenate(
            [w_attn[:, 2 * C + 64 * h:2 * C + 64 * h + 64] for h in heads], axis=1)
        wp = np.concatenate([w_proj[64 * h:64 * h + 64, :] for h in heads], axis=0)
        in_maps.append({
            "x": np.ascontiguousarray(x[b].T).astype(bf),
            "wqk": np.ascontiguousarray(wqk).astype(bf),
            "wv": np.ascontiguousarray(wv).astype(bf),
            "wp": np.ascontiguousarray(wp).astype(bf),
        })
    return in_maps


_NC_CACHE = None


def _get_program():
    global _NC_CACHE
    if _NC_CACHE is None:
        _NC_CACHE = build_program()
    return _NC_CACHE


def kernel(x, w_attn, w_proj, _run_kwargs=None):
    x = np.asarray(x, dtype=np.float32)
    w_attn = np.asarray(w_attn, dtype=np.float32)
    w_proj = np.asarray(w_proj, dtype=np.float32)
    nc = _get_program()
    in_maps = shard_inputs(x, w_attn, w_proj)
    res = run_bass_kernel_spmd(nc, in_maps, core_ids=list(range(N_CORES)),
                               **(_run_kwargs or {}))
    y = np.zeros((B, T, C), dtype=np.float32)
    for core in range(N_CORES):
        b = core // HEADS_PER_CORE
        y[b] += res.results[core]["out"]
    if _run_kwargs:
        return y, res
    return y
